# revision 1
# baseline (speedup 1.0000x reference)
"""Trainium2 Bass kernel for nn_BidirectionalAttention (B=16,H=4,T=256,N=2048,D=256).

Math (reference):
    Qr = rope2d(Q), Kr = rope2d(K)              # elementwise, per (t, n) angle
    scores = Qr @ Kr^T / sqrt(N)                # (B,H,T,T), no softmax
    out    = scores @ V                         # V (B,1,T,D) broadcasts over H

Kernel strategy (per core; B sharded 2-per-core across 8 cores):
  * RoPE is decomposed as Qr = A + pairswap(B) with A = Q*c, B = Q*s'
    where c, s' are host-precomputed tables (s' is the sine table
    pair-shuffled and sign-flipped so that pairswap(B) supplies the
    rotated term).  The 1/sqrt(N) is folded into the tables (N^-1/4 on
    both Q and K sides).
  * A and B are computed as two big bf16 DVE tensor_tensor multiplies
    (2x perf mode).  The pair swap AND the A+B add are folded into the
    PE transposes: transpose(A_blk) and transpose(B_blk with pair-swapped
    column access pattern) accumulate into the same PSUM tile, yielding
    QrT/KrT = (A + pairswap(B))^T directly.
  * mm1: scoresT[s,t] = sum_n KrT[n,s]^T-slice @ QrT[n,:]   (bf16, N=256)
  * mm2: out[t,d]     = sum_s scoresT[s][:,t_chunk] @ V[s]  (bf16, N=256)
  * ScalarE does the PSUM->SBUF copies; VectorE the RoPE multiplies;
    output copied f32 and DMAed out.
"""

import math
import os
import numpy as np
import ml_dtypes
from contextlib import ExitStack

import concourse.bass as bass
import concourse.bacc as bacc_mod
import concourse.tile as tile
import concourse.mybir as mybir
from concourse.bass_utils import run_bass_kernel_spmd
from concourse.masks import make_identity

bf16 = ml_dtypes.bfloat16

# problem shapes (hardcoded per contract)
B, H, T, N, D = 16, 4, 256, 2048, 256
GRID = 16
THETA = 10000.0
NCORES = 8
BS = B // NCORES          # batches per core
P = 128
NCH = N // P              # 16 feature chunks
TCH = T // P              # 2 token chunks

LAST_RESULT = None        # BassKernelResults of the most recent run (for test.py)


def _rope_tables():
    """Host-precomputed c and s' tables, bf16, in the SBUF tile layout
    [128, TCH*N] with [p, tc*N + n] = tab[tc*128 + p, n]; 1/sqrt(N) folded."""
    half = N // 2
    inv_freq = (1.0 / THETA ** (np.arange(0, half, 2, dtype=np.float32) / np.float32(half))).astype(np.float32)
    pos = np.arange(GRID, dtype=np.float32)
    ph = pos[:, None] * inv_freq[None, :]                      # (16, 512)
    ph_h = np.broadcast_to(ph[:, None, :], (GRID, GRID, half // 2))
    ph_w = np.broadcast_to(ph[None, :, :], (GRID, GRID, half // 2))
    phases = np.concatenate([ph_h, ph_w, ph_h, ph_w], axis=-1).reshape(GRID * GRID, N)
    ang = np.mod(phases, np.float32(1.0)) * np.float32(2.0 * math.pi)
    c = np.cos(ang).astype(np.float32)
    s = np.sin(ang).astype(np.float32)
    # s': s'[2i] = s[2i+1], s'[2i+1] = -s[2i]
    sp = np.empty_like(s)
    sp[:, 0::2] = s[:, 1::2]
    sp[:, 1::2] = -s[:, 0::2]
    alpha = np.float32(1.0 / math.sqrt(math.sqrt(N)))
    c *= alpha
    sp *= alpha
    # tile layout
    ct = np.empty((P, TCH * N), dtype=np.float32)
    st = np.empty((P, TCH * N), dtype=np.float32)
    for tc in range(TCH):
        ct[:, tc * N:(tc + 1) * N] = c[tc * P:(tc + 1) * P]
        st[:, tc * N:(tc + 1) * N] = sp[tc * P:(tc + 1) * P]
    return ct.astype(bf16), st.astype(bf16)


def _build_nc():
    nc = bacc_mod.Bacc("TRN2", target_bir_lowering=False, debug=False)

    q_dram = nc.dram_tensor("Q", [BS, H, T, N], mybir.dt.float32, kind="ExternalInput").ap()
    k_dram = nc.dram_tensor("K", [BS, H, T, N], mybir.dt.float32, kind="ExternalInput").ap()
    v_dram = nc.dram_tensor("V", [BS, 1, T, D], mybir.dt.float32, kind="ExternalInput").ap()
    c_dram = nc.dram_tensor("CT", [P, TCH * N], mybir.dt.bfloat16, kind="ExternalInput").ap()
    s_dram = nc.dram_tensor("ST", [P, TCH * N], mybir.dt.bfloat16, kind="ExternalInput").ap()
    o_dram = nc.dram_tensor("O", [BS, H, T, D], mybir.dt.float32, kind="ExternalOutput").ap()

    with tile.TileContext(nc) as tc, ExitStack() as ctx:
        const_pool = ctx.enter_context(tc.tile_pool(name="const", bufs=1))
        qk_pool = ctx.enter_context(tc.tile_pool(name="qk", bufs=2))
        v_pool = ctx.enter_context(tc.tile_pool(name="vp", bufs=2))
        ab_pool = ctx.enter_context(tc.tile_pool(name="ab", bufs=2))
        xt_pool = ctx.enter_context(tc.tile_pool(name="xt", bufs=2))
        sc_pool = ctx.enter_context(tc.tile_pool(name="scp", bufs=2))
        out_pool = ctx.enter_context(tc.tile_pool(name="outp", bufs=2))
        ps_tr = ctx.enter_context(tc.tile_pool(name="pstr", bufs=4, space="PSUM"))
        ps_sc = ctx.enter_context(tc.tile_pool(name="pssc", bufs=2, space="PSUM"))
        ps_out = ctx.enter_context(tc.tile_pool(name="psout", bufs=2, space="PSUM"))

        ident = const_pool.tile([P, P], mybir.dt.bfloat16)
        make_identity(nc, ident[:])
        ct = const_pool.tile([P, TCH * N], mybir.dt.bfloat16)
        st = const_pool.tile([P, TCH * N], mybir.dt.bfloat16)
        nc.sync.dma_start(ct[:], c_dram)
        nc.sync.dma_start(st[:], s_dram)
        # absorb each table-DMA wait into DVE engine order once, so the
        # per-(b,h) multiplies never need more than one sem wait (the
        # walrus TT encoding has a single sync-wait slot)
        scr1 = const_pool.tile([1, 8], mybir.dt.bfloat16)
        scr2 = const_pool.tile([1, 8], mybir.dt.bfloat16)
        nc.vector.tensor_copy(scr1[:], ct[0:1, 0:8])
        nc.vector.tensor_copy(scr2[:], st[0:1, 0:8])

        for b in range(BS):
            # V: DMA f32 then cast on ScalarE, so mm2's waits stay on the
            # Activation clock (single-wait-slot friendly)
            v_f32 = v_pool.tile([P, TCH * D], mybir.dt.float32, tag="vf32")
            nc.gpsimd.dma_start(
                v_f32[:].rearrange("p (tc d) -> p tc d", tc=TCH),
                v_dram[b, 0].rearrange("(tc p) d -> p tc d", p=P))
            v_bf = v_pool.tile([P, TCH * D], mybir.dt.bfloat16, tag="vbf")
            nc.scalar.copy(v_bf[:], v_f32[:])
            for h in range(H):
                # Per-t-chunk tiles and multiplies: shortens the critical
                # chain from last-DMA-byte to first/last PE work (pipeline
                # fill/drain), since deps are tracked per tile.
                kq_tiles = {}
                for (name, dram) in (("k", k_dram), ("q", q_dram)):
                    for tch in range(TCH):
                        tl = qk_pool.tile([P, N], mybir.dt.bfloat16,
                                          tag=f"{name}bf{tch}")
                        nc.gpsimd.dma_start(tl[:], dram[b, h][tch * P:(tch + 1) * P, :])
                        kq_tiles[(name, tch)] = tl
                ab_tiles = {}
                for (name, tab, tabname) in (("k", ct, "a"), ("k", st, "b"),
                                             ("q", ct, "a"), ("q", st, "b")):
                    for tch in range(TCH):
                        tl = ab_pool.tile([P, N], mybir.dt.bfloat16,
                                          tag=f"{tabname}{name}{tch}")
                        nc.vector.tensor_mul(tl[:], kq_tiles[(name, tch)][:],
                                             tab[:, tch * N:(tch + 1) * N])
                        ab_tiles[(tabname, name, tch)] = tl

                # Feature chunks use a de-interleaved order: chunk k = 2w+par
                # holds the parity-`par` columns of the 256-wide window w.
                # Under this order the RoPE pair-swap becomes an even<->odd
                # offset swap between the A and B transposes (single-free-dim
                # stride-2 APs, which the matmul RHS requires).  The same
                # order is used for Q and K, so scores are unchanged.
                qrT = xt_pool.tile([P, NCH * T], mybir.dt.bfloat16, tag="qrT")
                krT = xt_pool.tile([P, NCH * T], mybir.dt.bfloat16, tag="krT")
                # Transposes as REGULAR matmuls (out = blk^T @ I) so the PSUM
                # tile is fp32 and the A+B accumulate is exact (TRN2 PSUM
                # accumulation is fp32-only; bf16 transpose-mode accumulate
                # silently corrupts).
                # Two chunks share one [128,512] PSUM tile (one bank) so each
                # ScalarE copy moves 512 elems, amortizing its fixed overhead.
                # Interleave K/Q transpose groups and run mm1 on each chunk
                # pair as soon as both sides are copied — keeps the PE's
                # dependency chain on ScalarE short.
                # Transpose quads: one [128,512] fp32 PSUM tile = 4 chunks of
                # one t-chunk; 8 PE matmuls accumulate the A+B pairs, then one
                # wide ScalarE copy lands them in the (chunk-major) xT layout.
                sc_ps = []
                for sch in range(TCH):
                    sc_tile = ps_sc.tile([P, T], mybir.dt.float32, tag="scps")
                    sc_ps.append(sc_tile)
                for kk in range(0, NCH, 4):
                    for (name, xT) in (("k", krT), ("q", qrT)):
                        for tch in range(TCH):
                            a_t = ab_tiles[("a", name, tch)]
                            b_t = ab_tiles[("b", name, tch)]
                            pt = ps_tr.tile([P, 4 * P], mybir.dt.float32, tag="trps")
                            for j in range(4):
                                k = kk + j
                                w, par = divmod(k, 2)
                                base = w * 2 * P
                                a_blk = a_t[:, base + par: base + 2 * P: 2]
                                b_blk = b_t[:, base + (1 - par): base + 2 * P: 2]
                                dst = pt[:, j * P:(j + 1) * P]
                                nc.tensor.matmul(dst, a_blk, ident[:],
                                                 start=True, stop=False)
                                nc.tensor.matmul(dst, b_blk, ident[:],
                                                 start=False, stop=True)
                            dst_ap = xT[:].rearrange(
                                "p (k tc c) -> p k tc c", k=NCH, tc=TCH)[
                                :, kk:kk + 4, tch, :]
                            nc.scalar.copy(
                                dst_ap, pt[:].rearrange("p (j c) -> p j c", j=4))
                    # mm1 for this chunk quad
                    for n in range(kk, kk + 4):
                        for sch in range(TCH):
                            lhsT = krT[:, n * T + sch * P: n * T + (sch + 1) * P]
                            nc.tensor.matmul(sc_ps[sch][:], lhsT,
                                             qrT[:, n * T:(n + 1) * T],
                                             start=(n == 0), stop=(n == NCH - 1))
                sc_sb = sc_pool.tile([P, TCH * T], mybir.dt.bfloat16, tag="scsb")
                for sch in range(TCH):
                    nc.scalar.copy(sc_sb[:, sch * T:(sch + 1) * T], sc_ps[sch][:])

                # mm2: out[t_chunk] [128t, 256d] = sum_s scoresT[s][:,t_chunk] @ V[s]
                o_ps = ps_out.tile([P, TCH * D], mybir.dt.float32, tag="ops")
                for tch in range(TCH):
                    for sch in range(TCH):
                        lhsT = sc_sb[:, sch * T + tch * P: sch * T + (tch + 1) * P]
                        rhs = v_bf[:, sch * D:(sch + 1) * D]
                        nc.tensor.matmul(o_ps[:, tch * D:(tch + 1) * D], lhsT, rhs,
                                         start=(sch == 0), stop=(sch == TCH - 1))
                o_sb = out_pool.tile([P, TCH * D], mybir.dt.float32, tag="osb")
                nc.vector.tensor_copy(o_sb[:], o_ps[:])
                nc.sync.dma_start(
                    o_dram[b, h].rearrange("(tc p) d -> p tc d", p=P),
                    o_sb[:].rearrange("p (tc d) -> p tc d", tc=TCH))
    return nc


_NC_CACHE = None


def kernel(Q, K, V):
    global _NC_CACHE, LAST_RESULT
    Q = np.ascontiguousarray(np.asarray(Q, dtype=np.float32))
    K = np.ascontiguousarray(np.asarray(K, dtype=np.float32))
    V = np.ascontiguousarray(np.asarray(V, dtype=np.float32))
    assert Q.shape == (B, H, T, N) and K.shape == (B, H, T, N) and V.shape == (B, 1, T, D)

    if _NC_CACHE is None:
        _NC_CACHE = _build_nc()
        _NC_CACHE.compile()
    nc = _NC_CACHE
    ct, st = _rope_tables()

    in_maps = []
    for c in range(NCORES):
        sl = slice(c * BS, (c + 1) * BS)
        in_maps.append({"Q": Q[sl], "K": K[sl], "V": V[sl], "CT": ct, "ST": st})

    trace = bool(os.environ.get("BASS_KERNEL_TRACE"))
    res = run_bass_kernel_spmd(nc, in_maps, list(range(NCORES)), trace=trace,
                               trace_cores=[0] if trace else None)
    LAST_RESULT = res
    out = np.concatenate([res.results[c]["O"] for c in range(NCORES)], axis=0)
    return out

